# revision 1
# baseline (speedup 1.0000x reference)
"""Trainium2 Bass kernel for nn_Attention_85074712199827.

Computes, for hidden [1,32,1024], encoder_outputs [32,2048,1024],
W_attn [1024,2048], b_attn [1024], v [1024]:

    h_proj  = hidden[0] @ W_attn[:, :1024].T
    e_proj  = encoder_outputs @ W_attn[:, 1024:].T
    energy  = tanh(e_proj + h_proj[:, None, :] + b_attn)
    att     = energy @ v
    out     = softmax(att, axis=1)          # [32, 2048] float32

Distribution: data-parallel over the batch across 8 NeuronCores (4
batch rows per core); the tiny parameters are replicated (pre-laid-out
and pre-cast to bf16 on the host). Each core runs an independent
Bass/Tile program; results are concatenated on the host.

Self-contained: only environment packages (concourse, numpy, ml_dtypes)
are imported; all shapes/sharding are hardcoded for this problem.
"""

from contextlib import ExitStack

import ml_dtypes
import numpy as np

import concourse.bass as bass
import concourse.tile as tile
from concourse import bacc, mybir

F32 = mybir.dt.float32
BF16 = mybir.dt.bfloat16
AF = mybir.ActivationFunctionType
P = 128


def build_nc(b_loc=4, s=2048, h=1024, n_cores=8, sb=512,
             transpose_mode="sbuf", warmup_mm=32,
             pe_bufs=5, encT_bufs=5, inp_bufs=3, bfp_bufs=5,
             group_units=4, first_units=3, keepalive_mm=0, startup_keep=12):
    n_sb = s // sb          # s-blocks per batch
    n_hc = h // P           # contraction chunks
    n_ot = h // P           # output (o) tiles
    si_n = sb // P          # 128-row subtiles per s-block
    n_sc = sb // 512        # 512-wide psum chunks per s-block
    SC = 512

    nc = bacc.Bacc("TRN2", target_bir_lowering=False, debug=False,
                   num_devices=n_cores)

    wt = nc.dram_tensor("wt", [2 * h, h], BF16, kind="ExternalInput").ap()
    hiddenT = nc.dram_tensor("hiddenT", [h, b_loc], BF16, kind="ExternalInput").ap()
    b_attn = nc.dram_tensor("b_attn", [h], F32, kind="ExternalInput").ap()
    v = nc.dram_tensor("v", [h], BF16, kind="ExternalInput").ap()
    enc = nc.dram_tensor("enc", [b_loc, s, h], F32, kind="ExternalInput").ap()
    out = nc.dram_tensor("out", [b_loc, s], F32, kind="ExternalOutput").ap()

    with tile.TileContext(nc) as tc, ExitStack() as ctx:
        const = ctx.enter_context(tc.tile_pool(name="const", bufs=1))
        psmall = ctx.enter_context(tc.tile_pool(name="psmall", bufs=1, space="PSUM"))

        # ---- PE warmup: dependency-free matmuls to lift the HAM clock
        # gate to 8/8 while the first enc block is still in flight ----
        if warmup_mm:
            wz = const.tile([P, SC], BF16)
            nc.gpsimd.memset(wz[:], 0)
            for i in range(warmup_mm):
                pw = psmall.tile([P, SC], F32, name="pw", tag="ps")
                nc.tensor.matmul(pw[:], wz[:, :P], wz[:], start=True, stop=True)

        # ---- small constants first (tiny; keep them off the critical
        # xbar-drain path) ----
        hT_bf = const.tile([P, n_hc, b_loc], BF16)
        nc.scalar.dma_start(hT_bf[:], hiddenT.rearrange("(hc p) b -> p hc b", p=P))

        baT = const.tile([P, n_ot], F32)
        nc.scalar.dma_start(baT[:], b_attn.rearrange("(oc p) -> p oc", p=P))

        vt_bf = const.tile([P, n_ot], BF16)
        nc.scalar.dma_start(vt_bf[:], v.rearrange("(oc p) -> p oc", p=P))

        # ---- weights: W_attn.T arrives [2h, h] bf16; Wh half first so
        # h_proj unblocks while We still streams ----
        wt_bf = const.tile([P, 2 * n_hc, h], BF16)
        wt_r = wt.rearrange("(jc p) o -> p jc o", p=P)
        q = n_hc // 2

        def emit_w(c):
            nc.sync.dma_start(
                wt_bf[:, c * q:(c + 1) * q, :],
                wt_r[:, c * q:(c + 1) * q, :])

        emit_w(2)
        emit_w(3)

        def emit_hproj():
            hb = const.tile([P, n_ot, b_loc], F32, name="hb")
            for ot in range(n_ot):
                ph = psmall.tile([P, b_loc], F32, name="ph", tag="ps")
                for hc in range(n_hc):
                    nc.tensor.matmul(
                        ph[:], wt_bf[:, hc, ot * P:(ot + 1) * P], hT_bf[:, hc, :],
                        start=(hc == 0), stop=(hc == n_hc - 1))
                nc.vector.tensor_tensor(
                    hb[:, ot, :], ph[:],
                    baT[:, ot, None].to_broadcast((P, b_loc)),
                    mybir.AluOpType.add)
            return hb

        # ---- main pipeline pools ----
        inp = ctx.enter_context(tc.tile_pool(name="inp", bufs=inp_bufs))
        bfp = ctx.enter_context(tc.tile_pool(name="bfp", bufs=bfp_bufs))
        if transpose_mode == "dram":
            dram = ctx.enter_context(tc.tile_pool(name="dram", bufs=4, space="DRAM"))
        encT_p = ctx.enter_context(tc.tile_pool(name="encT", bufs=encT_bufs))
        en_p = ctx.enter_context(tc.tile_pool(name="energy", bufs=3))
        pe_p = ctx.enter_context(tc.tile_pool(name="psum_e", bufs=pe_bufs, space="PSUM"))
        pa_p = ctx.enter_context(tc.tile_pool(name="psum_att", bufs=2, space="PSUM"))

        att_rows = const.tile([b_loc, s], F32)

        units = [(b, isb) for b in range(b_loc) for isb in range(n_sb)]

        def phase1(unit):
            b, isb = unit
            sl = slice(isb * sb, (isb + 1) * sb)
            it = inp.tile([P, si_n, h], F32, name="it")
            nc.sync.dma_start(
                it[:], enc[b, sl, :].rearrange("(si p) h -> p si h", p=P))
            bt = bfp.tile([P, si_n, h], BF16, name="bt")
            nc.vector.tensor_copy(out=bt[:], in_=it[:])
            return bt

        def phase2(bt):
            eT = encT_p.tile([P, n_hc, sb], BF16, name="eT")
            for si in range(si_n):
                nc.sync.dma_start_transpose(
                    eT[:, :, si * P:(si + 1) * P], bt[:, si, :])
            return eT

        def phase3(unit, eT, hb):
            b, isb = unit
            sl = slice(isb * sb, (isb + 1) * sb)
            pa_full = pa_p.tile([P, sb], F32, name="pa")
            pa = pa_full[0:1, :]
            pending = None  # v-dot lags one ot-group so tanh is long done
            for ot in range(n_ot):
                for sc in range(n_sc):
                    scl = slice(sc * SC, (sc + 1) * SC)
                    pe = pe_p.tile([P, SC], F32, name="pe")
                    for hc in range(n_hc):
                        nc.tensor.matmul(
                            pe[:], wt_bf[:, n_hc + hc, ot * P:(ot + 1) * P],
                            eT[:, hc, scl],
                            start=(hc == 0), stop=(hc == n_hc - 1))
                    eng = en_p.tile([P, SC], BF16, name="eng")
                    nc.scalar.activation(
                        eng[:], pe[:], AF.Tanh, bias=hb[:, ot, b:b + 1])
                    if pending is not None:
                        pot, peng, pscl = pending
                        nc.tensor.matmul(
                            pa[0:1, pscl], vt_bf[:, pot:pot + 1], peng[:],
                            start=(pot == 0), stop=False,
                            skip_group_check=True)
                    pending = (ot, eng, scl)
            pot, peng, pscl = pending
            nc.tensor.matmul(
                pa[0:1, pscl], vt_bf[:, pot:pot + 1], peng[:],
                start=(pot == 0), stop=True,
                skip_group_check=True)
            att_sb = en_p.tile([1, sb], F32, name="att_sb")
            nc.scalar.activation(att_sb[:], pa[:], AF.Copy)
            nc.gpsimd.dma_start(att_rows[b:b + 1, sl], att_sb[:])

        def keepalive(n):
            for _ in range(n):
                pw = psmall.tile([P, SC], F32, name="pw", tag="ps")
                nc.tensor.matmul(pw[:], wz[:, :P], wz[:], start=True, stop=True)

        # staged startup: u0 alone (smallest xbar-drain set), then u1-2,
        # then steady-state groups; We and h_proj interleave so the PE
        # stream has no hole wider than the HAM window
        bt0 = phase1(units[0])
        eT0 = phase2(bt0)
        emit_w(0)
        emit_w(1)
        hb = emit_hproj()
        phase3(units[0], eT0, hb)
        keepalive(startup_keep)

        rest = units[3:]
        groups = [rest[i:i + group_units]
                  for i in range(0, len(rest), group_units)]

        # software-pipelined: group g's transposes run first, then group
        # g+1's plain copies stream while group g's matmuls execute — the
        # xbar-mode drain pairs (copies <-> transposes) never block the PE
        mid = units[1:3]
        bt12 = [phase1(u) for u in mid]
        eT12 = [phase2(bt) for bt in bt12]
        bts_next = [phase1(u) for u in groups[0]] if groups else []
        for u, eT in zip(mid, eT12):
            phase3(u, eT, hb)
        keepalive(startup_keep)

        for gi, group in enumerate(groups):
            eTs = [phase2(bt) for bt in bts_next]
            if gi + 1 < len(groups):
                bts_next = [phase1(u) for u in groups[gi + 1]]
            for u, eT in zip(group, eTs):
                phase3(u, eT, hb)
            keepalive(keepalive_mm)

        # ---- softmax over s per batch row ----
        mneg = const.tile([b_loc, 1], F32)
        nc.vector.tensor_reduce(
            mneg[:], att_rows[:], mybir.AxisListType.X, mybir.AluOpType.max)
        nc.vector.tensor_scalar_mul(mneg[:], mneg[:], -1.0)
        e_rows = const.tile([b_loc, s], F32)
        ssum = const.tile([b_loc, 1], F32)
        nc.scalar.activation(
            e_rows[:], att_rows[:], AF.Exp, bias=mneg[:], accum_out=ssum[:])
        rinv = const.tile([b_loc, 1], F32)
        nc.vector.reciprocal(rinv[:], ssum[:])
        o_rows = const.tile([b_loc, s], F32)
        nc.vector.tensor_scalar_mul(o_rows[:], e_rows[:], rinv[:])
        nc.sync.dma_start(out[:, :], o_rows[:])

    nc.compile()
    return nc


def make_in_maps(hidden, encoder_outputs, W_attn, b_attn, v, n_cores=8):
    hidden = np.asarray(hidden, dtype=np.float32)
    encoder_outputs = np.asarray(encoder_outputs, dtype=np.float32)
    W_attn = np.asarray(W_attn, dtype=np.float32)
    b_attn = np.asarray(b_attn, dtype=np.float32)
    v = np.asarray(v, dtype=np.float32)

    b = encoder_outputs.shape[0]
    b_loc = b // n_cores
    wt = np.ascontiguousarray(W_attn.T.astype(ml_dtypes.bfloat16))
    v_bf = v.astype(ml_dtypes.bfloat16)
    in_maps = []
    for i in range(n_cores):
        bsl = slice(b_loc * i, b_loc * (i + 1))
        in_maps.append({
            "wt": wt,
            "hiddenT": np.ascontiguousarray(
                hidden[0, bsl].T.astype(ml_dtypes.bfloat16)),
            "b_attn": b_attn,
            "v": v_bf,
            "enc": np.ascontiguousarray(encoder_outputs[bsl]),
        })
    return in_maps


_NC_CACHE = {}


def _get_nc():
    if "nc" not in _NC_CACHE:
        _NC_CACHE["nc"] = build_nc(b_loc=4, s=2048, h=1024, n_cores=8)
    return _NC_CACHE["nc"]


def kernel(hidden, encoder_outputs, W_attn, b_attn, v):
    from concourse.bass_utils import run_bass_kernel_spmd

    nc = _get_nc()
    in_maps = make_in_maps(hidden, encoder_outputs, W_attn, b_attn, v,
                           n_cores=8)
    res = run_bass_kernel_spmd(nc, in_maps, core_ids=list(range(8)))
    out = np.concatenate([np.asarray(res.results[i]["out"])
                          for i in range(8)], axis=0)
    return out.astype(np.float32)



# revision 4
# speedup vs baseline: 1.2498x; 1.2498x over previous
"""Trainium2 Bass kernel for nn_Attention_85074712199827.

Computes, for hidden [1,32,1024], encoder_outputs [32,2048,1024],
W_attn [1024,2048], b_attn [1024], v [1024]:

    h_proj  = hidden[0] @ W_attn[:, :1024].T
    e_proj  = encoder_outputs @ W_attn[:, 1024:].T
    energy  = tanh(e_proj + h_proj[:, None, :] + b_attn)
    att     = energy @ v
    out     = softmax(att, axis=1)          # [32, 2048] float32

Distribution: data-parallel over the batch across 8 NeuronCores (4
batch rows per core); parameters replicated. All operands are pre-laid
out on the host: W_attn.T and hidden.T in bf16, and encoder_outputs
pre-transposed per batch row to [h, s] bf16 so the device needs no
on-chip transposes or casts — the PE consumes DMA-ed tiles directly.

Per core the PE stream is: a short back-to-back warmup burst (trips the
HAM clock gate to 8/8), h_proj (64 small matmuls), then 16 units
(4 batch rows x 4 s-chunks of 512) of 64 e_proj matmuls each. The
v-weighted reduction over the hidden axis runs on the vector engine
(one fused multiply-add per 128-chunk) followed by a single ones-vector
matmul per unit; softmax is per batch row on partition 0.

Self-contained: only environment packages (concourse, numpy, ml_dtypes)
are imported; all shapes/sharding are hardcoded for this problem.
"""

from contextlib import ExitStack

import ml_dtypes
import numpy as np

import concourse.bass as bass  # noqa: F401  (namespace import keeps parity with env)
import concourse.tile as tile
from concourse import bacc, mybir

F32 = mybir.dt.float32
BF16 = mybir.dt.bfloat16
AF = mybir.ActivationFunctionType
ALU = mybir.AluOpType
P = 128

PROBE_VALS = [-5.0, -10.0, -15.0, -20.0, -30.0, -40.0, -60.0, -80.0]


def build_nc(b_loc=4, s=2048, h=1024, n_cores=8,
             warm_a=16, warm_b=4, enc_bufs=4, pe_bufs=5, eng_bufs=4):
    SC = 512                 # s-chunk width (one PSUM bank of f32)
    n_sc = s // SC           # s-chunks per batch row
    n_hc = h // P            # contraction chunks
    n_ot = h // P            # output (o) tiles

    nc = bacc.Bacc("TRN2", target_bir_lowering=False, debug=False,
                   num_devices=n_cores)

    wt = nc.dram_tensor("wt", [2 * h, h], BF16, kind="ExternalInput").ap()
    hiddenT = nc.dram_tensor("hiddenT", [h, b_loc], BF16, kind="ExternalInput").ap()
    b_attn = nc.dram_tensor("b_attn", [h], F32, kind="ExternalInput").ap()
    v = nc.dram_tensor("v", [h], F32, kind="ExternalInput").ap()
    encT = nc.dram_tensor("encT", [b_loc, h, s], BF16, kind="ExternalInput").ap()
    probe = nc.dram_tensor("probe", [1, 8], F32, kind="ExternalInput").ap()
    out = nc.dram_tensor("out", [b_loc, s], F32, kind="ExternalOutput").ap()
    dbg = nc.dram_tensor("dbg", [1, 8], F32, kind="ExternalOutput").ap()

    with tile.TileContext(nc) as tc, ExitStack() as ctx:
        const = ctx.enter_context(tc.tile_pool(name="const", bufs=1))
        pe_p = ctx.enter_context(tc.tile_pool(name="pe", bufs=pe_bufs, space="PSUM"))
        pa_p = ctx.enter_context(tc.tile_pool(name="pa", bufs=2, space="PSUM"))
        ps_p = ctx.enter_context(tc.tile_pool(name="ps", bufs=1, space="PSUM"))
        encp = ctx.enter_context(tc.tile_pool(name="encp", bufs=enc_bufs))
        engp = ctx.enter_context(tc.tile_pool(name="engp", bufs=eng_bufs))
        accp = ctx.enter_context(tc.tile_pool(name="accp", bufs=2))
        accbp = ctx.enter_context(tc.tile_pool(name="accbp", bufs=2))

        # ---- zeros for warmup; ones column for the partition-reduce ----
        wz = const.tile([P, SC], BF16)
        nc.gpsimd.memset(wz[:], 0)
        ones_bf = const.tile([P, 1], BF16)
        nc.gpsimd.memset(ones_bf[:], 1.0)

        def warm(n):
            # independent back-to-back matmuls cycling the pe pool: a
            # gapless PE burst (a semaphore-serialized chain never trips
            # the HAM activity window — it needs contiguous busy time)
            for _ in range(n):
                pw = pe_p.tile([P, SC], F32, name="pe")
                nc.tensor.matmul(pw[:], wz[:, :P], wz[:], start=True, stop=True)

        warm(warm_a)

        # ---- small constants (scalar HWDGE queue) ----
        hT_bf = const.tile([P, n_hc, b_loc], BF16)
        nc.scalar.dma_start(hT_bf[:], hiddenT.rearrange("(hc p) b -> p hc b", p=P))
        baT = const.tile([P, n_ot], F32)
        nc.scalar.dma_start(baT[:], b_attn.rearrange("(oc p) -> p oc", p=P))
        vT = const.tile([P, n_ot], F32)
        nc.scalar.dma_start(vT[:], v.rearrange("(oc p) -> p oc", p=P))
        probe_t = const.tile([1, 8], F32)
        nc.scalar.dma_start(probe_t[:], probe)

        # ---- weights: enc unit 0 rides first on the sync queue (see
        # below); Wh on sync next (h_proj), We streams on scalar ----
        wt_bf = const.tile([P, 2 * n_hc, h], BF16)
        wt_r = wt.rearrange("(jc p) o -> p jc o", p=P)

        att_rows = [const.tile([1, s], F32, name=f"att_r{i}")
                    for i in range(b_loc)]
        e_rows = [const.tile([1, s], F32, name=f"e_r{i}")
                  for i in range(b_loc)]
        pm0 = const.tile([1, b_loc * n_sc], F32)   # per-unit max partials
        hb = const.tile([P, n_ot, b_loc], F32)

        units = [(b, c) for b in range(b_loc) for c in range(n_sc)]

        def load_unit(b, c):
            it = encp.tile([P, n_hc, SC], BF16, name="it")
            nc.sync.dma_start(
                it[:],
                encT[b, :, c * SC:(c + 1) * SC].rearrange(
                    "(hc p) s -> p hc s", p=P))
            return it

        it0 = load_unit(*units[0])
        nc.sync.dma_start(wt_bf[:, 0:n_hc, :], wt_r[:, 0:n_hc, :])      # Wh
        for cch in range(n_hc):                                          # We
            nc.scalar.dma_start(wt_bf[:, n_hc + cch, :],
                                wt_r[:, n_hc + cch, :])

        # ---- h_proj: hb[:, ot, b] = Wh.T chunk @ hT + b_attn ----
        for ot in range(n_ot):
            ph = ps_p.tile([P, b_loc], F32, name="ph")
            for hc in range(n_hc):
                nc.tensor.matmul(
                    ph[:], wt_bf[:, hc, ot * P:(ot + 1) * P], hT_bf[:, hc, :],
                    start=(hc == 0), stop=(hc == n_hc - 1))
            nc.vector.tensor_tensor(
                hb[:, ot, :], ph[:],
                baT[:, ot, None].to_broadcast((P, b_loc)),
                ALU.add)

        warm(warm_b)

        # ---- exp-table probe (negligible; feeds a host-side check) ----
        dbg_t = const.tile([1, 8], F32)
        nc.scalar.activation(dbg_t[:], probe_t[:], AF.Exp)
        nc.gpsimd.dma_start(dbg, dbg_t[:])

        def emit_ones(pending):
            b, c, accb = pending
            pa = pa_p.tile([P, SC], F32, name="pa")
            nc.tensor.matmul(pa[0:1, :], ones_bf[:], accb[:],
                             start=True, stop=True)
            att_sb = engp.tile([1, SC], F32, name="att_sb")
            nc.scalar.activation(att_sb[:], pa[0:1, :], AF.Copy)
            nc.vector.tensor_copy(
                out=att_rows[b][:, c * SC:(c + 1) * SC], in_=att_sb[:])
            u = b * n_sc + c
            nc.vector.tensor_reduce(
                pm0[:, u:u + 1], att_sb[:], mybir.AxisListType.X, ALU.max)

        def run_unit(b, c, it, pending):
            acc = accp.tile([P, SC], F32, name="acc")
            for ot in range(n_ot):
                pe = pe_p.tile([P, SC], F32, name="pe")
                for hc in range(n_hc):
                    nc.tensor.matmul(
                        pe[:], wt_bf[:, n_hc + hc, ot * P:(ot + 1) * P],
                        it[:, hc, :],
                        start=(hc == 0), stop=(hc == n_hc - 1))
                eng = engp.tile([P, SC], BF16, name="eng")
                nc.scalar.activation(eng[:], pe[:], AF.Tanh,
                                     bias=hb[:, ot, b:b + 1])
                if ot == 0:
                    nc.vector.tensor_scalar(
                        acc[:], eng[:], vT[:, 0:1], None, ALU.mult)
                    # v-dot of the previous unit lags one ot-group so its
                    # accumulator is long finished when the PE reaches it
                    if pending is not None:
                        emit_ones(pending)
                else:
                    nc.vector.scalar_tensor_tensor(
                        acc[:], eng[:], vT[:, ot:ot + 1], acc[:],
                        ALU.mult, ALU.add)
            accb = accbp.tile([P, SC], BF16, name="accb")
            nc.vector.tensor_copy(out=accb[:], in_=acc[:])
            return (b, c, accb)

        def softmax_b(b):
            # all on partition 0: combine the 4 chunk maxes, exp with
            # running-sum, normalize, store
            mneg = const.tile([1, 1], F32)
            nc.vector.tensor_reduce(
                mneg[:], pm0[:, b * n_sc:(b + 1) * n_sc],
                mybir.AxisListType.X, ALU.max, negate=True)
            ssum = const.tile([1, 1], F32)
            nc.scalar.activation(e_rows[b][:], att_rows[b][:], AF.Exp,
                                 bias=mneg[:], accum_out=ssum[:])
            rinv = const.tile([1, 1], F32)
            nc.vector.reciprocal(rinv[:], ssum[:])
            nc.vector.tensor_scalar(
                e_rows[b][:], e_rows[b][:], rinv[:], None, ALU.mult)
            nc.sync.dma_start(out[b:b + 1, :], e_rows[b][:])

        loaded = {0: it0}
        pending = None
        for idx, (b, c) in enumerate(units):
            for j in range(idx + 1, min(idx + enc_bufs, len(units))):
                if j not in loaded:
                    loaded[j] = load_unit(*units[j])
            pending = run_unit(b, c, loaded.pop(idx), pending)
            if idx >= 1 and units[idx - 1][1] == n_sc - 1:
                softmax_b(units[idx - 1][0])
        emit_ones(pending)
        softmax_b(b_loc - 1)

    nc.compile()
    return nc


def make_in_maps(hidden, encoder_outputs, W_attn, b_attn, v, n_cores=8):
    hidden = np.asarray(hidden, dtype=np.float32)
    encoder_outputs = np.asarray(encoder_outputs, dtype=np.float32)
    W_attn = np.asarray(W_attn, dtype=np.float32)
    b_attn = np.asarray(b_attn, dtype=np.float32)
    v = np.asarray(v, dtype=np.float32)

    b = encoder_outputs.shape[0]
    b_loc = b // n_cores
    wt = np.ascontiguousarray(W_attn.T.astype(ml_dtypes.bfloat16))
    probe = np.array([PROBE_VALS], dtype=np.float32)
    in_maps = []
    for i in range(n_cores):
        bsl = slice(b_loc * i, b_loc * (i + 1))
        in_maps.append({
            "wt": wt,
            "hiddenT": np.ascontiguousarray(
                hidden[0, bsl].T.astype(ml_dtypes.bfloat16)),
            "b_attn": b_attn,
            "v": v,
            "encT": np.ascontiguousarray(
                encoder_outputs[bsl].transpose(0, 2, 1)
                .astype(ml_dtypes.bfloat16)),
            "probe": probe,
        })
    return in_maps


_NC_CACHE = {}


def _get_nc():
    if "nc" not in _NC_CACHE:
        _NC_CACHE["nc"] = build_nc(b_loc=4, s=2048, h=1024, n_cores=8)
    return _NC_CACHE["nc"]


def kernel(hidden, encoder_outputs, W_attn, b_attn, v):
    from concourse.bass_utils import run_bass_kernel_spmd

    nc = _get_nc()
    in_maps = make_in_maps(hidden, encoder_outputs, W_attn, b_attn, v,
                           n_cores=8)
    res = run_bass_kernel_spmd(nc, in_maps, core_ids=list(range(8)))
    out = np.concatenate([np.asarray(res.results[i]["out"])
                          for i in range(8)], axis=0)
    return out.astype(np.float32)


# revision 8
# speedup vs baseline: 1.2984x; 1.0389x over previous
"""Trainium2 Bass kernel for nn_Attention_85074712199827.

Computes, for hidden [1,32,1024], encoder_outputs [32,2048,1024],
W_attn [1024,2048], b_attn [1024], v [1024]:

    h_proj  = hidden[0] @ W_attn[:, :1024].T
    e_proj  = encoder_outputs @ W_attn[:, 1024:].T
    energy  = tanh(e_proj + h_proj[:, None, :] + b_attn)
    att     = energy @ v
    out     = softmax(att, axis=1)          # [32, 2048] float32

Distribution: data-parallel over the batch across 8 NeuronCores (4
batch rows per core); parameters replicated. All operands are pre-laid
out on the host: W_attn.T and hidden.T in bf16, and encoder_outputs
pre-transposed per batch row to [h, s] bf16, so the device needs no
on-chip transposes or casts — the PE consumes DMA-ed tiles directly.

PE stream: a short back-to-back warmup burst (trips the HAM clock gate
to 8/8 while the first DMAs land), then 16 units (4 batch rows x 4
s-chunks of 512) of 64 e_proj matmuls each; h_proj's 64 small matmuls
are spliced into unit 0 after its 6th group so the PE never waits on
the Wh weights. The v-weighted hidden-axis reduction runs on the
vector engine (one fused multiply-add per 128-chunk) plus a single
ones-vector matmul per unit. Softmax uses a shift by the bound
sum(|v|) >= |att| instead of the max, so exp and the normalization sum
run per-unit, fully overlapped; only the last row's normalize+store
remains in the tail.

Self-contained: only environment packages (concourse, numpy, ml_dtypes)
are imported; all shapes/sharding are hardcoded for this problem.
"""

from contextlib import ExitStack

import ml_dtypes
import numpy as np

import concourse.bass as bass  # noqa: F401  (namespace import keeps parity with env)
import concourse.tile as tile
from concourse import bacc, mybir

F32 = mybir.dt.float32
BF16 = mybir.dt.bfloat16
AF = mybir.ActivationFunctionType
ALU = mybir.AluOpType
P = 128

PROBE_VALS = [-5.0, -10.0, -15.0, -20.0, -30.0, -40.0, -60.0, -80.0]


def build_nc(b_loc=4, s=2048, h=1024, n_cores=8,
             warm_a=6, enc_bufs=4, pe_bufs=6, eng_bufs=4, hp_at=6):
    SC = 512                 # s-chunk width (one PSUM bank of f32)
    n_sc = s // SC           # s-chunks per batch row
    n_hc = h // P            # contraction chunks
    n_ot = h // P            # output (o) tiles

    nc = bacc.Bacc("TRN2", target_bir_lowering=False, debug=False,
                   num_devices=n_cores)

    wt = nc.dram_tensor("wt", [2 * h, h], BF16, kind="ExternalInput").ap()
    hiddenT = nc.dram_tensor("hiddenT", [h, b_loc], BF16, kind="ExternalInput").ap()
    b_attn = nc.dram_tensor("b_attn", [h], F32, kind="ExternalInput").ap()
    v = nc.dram_tensor("v", [h], F32, kind="ExternalInput").ap()
    encT = nc.dram_tensor("encT", [b_loc, h, s], BF16, kind="ExternalInput").ap()
    probe = nc.dram_tensor("probe", [1, 8], F32, kind="ExternalInput").ap()
    nb = nc.dram_tensor("nb", [1, 1], F32, kind="ExternalInput").ap()
    out = nc.dram_tensor("out", [b_loc, s], F32, kind="ExternalOutput").ap()
    dbg = nc.dram_tensor("dbg", [1, 8], F32, kind="ExternalOutput").ap()

    wt_r = wt.rearrange("(jc p) o -> p jc o", p=P)

    with tile.TileContext(nc) as tc, ExitStack() as ctx:
        const = ctx.enter_context(tc.tile_pool(name="const", bufs=1))
        pe_p = ctx.enter_context(tc.tile_pool(name="pe", bufs=pe_bufs, space="PSUM"))
        pa_p = ctx.enter_context(tc.tile_pool(name="pa", bufs=1, space="PSUM"))
        ps_p = ctx.enter_context(tc.tile_pool(name="ps", bufs=1, space="PSUM"))
        encp = ctx.enter_context(tc.tile_pool(name="encp", bufs=enc_bufs))
        engp = ctx.enter_context(tc.tile_pool(name="engp", bufs=eng_bufs))
        accp = ctx.enter_context(tc.tile_pool(name="accp", bufs=2))

        # ---- zeros for warmup; ones column for the partition-reduce ----
        wz = const.tile([P, SC], BF16)
        nc.gpsimd.memset(wz[:], 0)
        ones_bf = const.tile([P, 1], BF16)
        nc.gpsimd.memset(ones_bf[:], 1.0)

        def warm(n):
            # independent back-to-back matmuls cycling the pe pool: a
            # gapless PE burst (a semaphore-serialized chain never trips
            # the HAM activity window — it needs contiguous busy time)
            for _ in range(n):
                pw = pe_p.tile([P, SC], F32, name="pe")
                nc.tensor.matmul(pw[:], wz[:, :P], wz[:], start=True, stop=True)

        warm(warm_a)

        # ---- small constants (scalar HWDGE queue) ----
        hT_bf = const.tile([P, n_hc, b_loc], BF16)
        nc.scalar.dma_start(hT_bf[:], hiddenT.rearrange("(hc p) b -> p hc b", p=P))
        baT = const.tile([P, n_ot], F32)
        nc.scalar.dma_start(baT[:], b_attn.rearrange("(oc p) -> p oc", p=P))
        vT = const.tile([P, n_ot], F32)
        nc.scalar.dma_start(vT[:], v.rearrange("(oc p) -> p oc", p=P))
        probe_t = const.tile([1, 8], F32)
        nc.scalar.dma_start(probe_t[:], probe)
        nb_t = const.tile([1, 1], F32)
        nc.scalar.dma_start(nb_t[:], nb)

        # ---- critical-order sync queue: We chunks and unit-0 enc chunks
        # interleaved (everything the first matmul group needs lands
        # first), then Wh, then the remaining enc units ----
        wt_bf = const.tile([P, 2 * n_hc, h], BF16)
        it0 = encp.tile([P, n_hc, SC], BF16, name="it")
        enc0_r = encT[0, :, 0:SC].rearrange("(hc p) s -> p hc s", p=P)
        for cch in range(n_hc):
            nc.sync.dma_start(wt_bf[:, n_hc + cch, :], wt_r[:, n_hc + cch, :])
            nc.sync.dma_start(it0[:, cch, :], enc0_r[:, cch, :])
        nc.sync.dma_start(wt_bf[:, 0:n_hc, :], wt_r[:, 0:n_hc, :])      # Wh

        e_rows = [const.tile([1, s], F32, name=f"e_r{i}")
                  for i in range(b_loc)]
        ssc = const.tile([1, b_loc * n_sc], F32)   # per-unit exp partial sums
        hb = const.tile([P, n_ot, b_loc], F32)

        units = [(b, c) for b in range(b_loc) for c in range(n_sc)]

        def load_unit(b, c):
            it = encp.tile([P, n_hc, SC], BF16, name="it")
            nc.sync.dma_start(
                it[:],
                encT[b, :, c * SC:(c + 1) * SC].rearrange(
                    "(hc p) s -> p hc s", p=P))
            return it

        def h_proj():
            # hb[:, ot, b] = (Wh.T chunk @ hT)[o, b] + b_attn[o]; the
            # bias-add rides the scalar engine (Identity + per-partition
            # bias) — it must precede every tanh in the ACT FIFO
            for ot in range(n_ot):
                ph = ps_p.tile([P, b_loc], F32, name="ph")
                for hc in range(n_hc):
                    nc.tensor.matmul(
                        ph[:], wt_bf[:, hc, ot * P:(ot + 1) * P],
                        hT_bf[:, hc, :],
                        start=(hc == 0), stop=(hc == n_hc - 1))
                nc.scalar.add(hb[:, ot, :], ph[:], baT[:, ot, None])

        # ---- exp-table probe (negligible; feeds a host-side check) ----
        dbg_t = const.tile([1, 8], F32)
        nc.scalar.activation(dbg_t[:], probe_t[:], AF.Exp)
        nc.gpsimd.dma_start(dbg, dbg_t[:])

        def emit_ones(pending):
            # att chunk = ones.T @ accb; then exp with the sum(|v|)-bound
            # shift straight out of PSUM, with a running per-chunk sum
            b, c, accb = pending
            u = b * n_sc + c
            pa = pa_p.tile([P, SC], F32, name="pa")
            nc.tensor.matmul(pa[0:1, :], ones_bf[:], accb[:],
                             start=True, stop=True)
            nc.scalar.activation(
                e_rows[b][:, c * SC:(c + 1) * SC], pa[0:1, :], AF.Exp,
                bias=nb_t[:], accum_out=ssc[:, u:u + 1])

        def mm_group(it, ot):
            pe = pe_p.tile([P, SC], F32, name="pe")
            for hc in range(n_hc):
                nc.tensor.matmul(
                    pe[:], wt_bf[:, n_hc + hc, ot * P:(ot + 1) * P],
                    it[:, hc, :],
                    start=(hc == 0), stop=(hc == n_hc - 1))
            return pe

        def tanh_fma(b, pe, ot, acc, accb, pending):
            eng = engp.tile([P, SC], BF16, name="eng")
            nc.scalar.activation(eng[:], pe[:], AF.Tanh,
                                 bias=hb[:, ot, b:b + 1])
            if ot == 0:
                nc.vector.tensor_scalar(
                    acc[:], eng[:], vT[:, 0:1], None, ALU.mult)
                # v-dot of the previous unit lags one ot-group so its
                # accumulator is long finished when the PE reaches it
                if pending is not None:
                    emit_ones(pending)
            else:
                nc.vector.scalar_tensor_tensor(
                    accb[:] if ot == n_ot - 1 else acc[:],
                    eng[:], vT[:, ot:ot + 1], acc[:],
                    ALU.mult, ALU.add)

        def run_unit(b, c, it, pending, hp=False):
            acc = accp.tile([P, SC], F32, name="acc")
            accb = accp.tile([P, SC], BF16, name="accb")
            if hp:
                # unit 0: run the first hp_at matmul groups WITHOUT their
                # tanhs (PSUM banks hold them), then h_proj, then drain.
                # Every hb producer thereby precedes every tanh in the
                # ACT FIFO, and the PE never waits on the Wh DMA.
                pes = [mm_group(it, ot) for ot in range(hp_at)]
                h_proj()
                for ot, pe in enumerate(pes):
                    tanh_fma(b, pe, ot, acc, accb, pending)
                start = hp_at
            else:
                start = 0
            for ot in range(start, n_ot):
                pe = mm_group(it, ot)
                tanh_fma(b, pe, ot, acc, accb, pending)
            return (b, c, accb)

        def softmax_b(b):
            # all on partition 0: total = sum of the 4 chunk sums, then
            # normalize in halves on two engines and store
            ssum = const.tile([1, 1], F32, name=f"ssum{b}")
            nc.vector.tensor_reduce(
                ssum[:], ssc[:, b * n_sc:(b + 1) * n_sc],
                mybir.AxisListType.X, ALU.add)
            rinv = const.tile([1, 1], F32, name=f"rinv{b}")
            nc.vector.reciprocal(rinv[:], ssum[:])
            half = s // 2
            nc.vector.tensor_scalar(
                e_rows[b][:, 0:half], e_rows[b][:, 0:half], rinv[:],
                None, ALU.mult)
            nc.scalar.mul(e_rows[b][:, half:s], e_rows[b][:, half:s],
                          rinv[:])
            nc.scalar.dma_start(out[b:b + 1, :], e_rows[b][:])

        loaded = {0: it0}
        pending = None
        for idx, (b, c) in enumerate(units):
            for j in range(idx + 1, min(idx + enc_bufs, len(units))):
                if j not in loaded:
                    loaded[j] = load_unit(*units[j])
            pending = run_unit(b, c, loaded.pop(idx), pending, hp=(idx == 0))
            if idx >= 1 and units[idx - 1][1] == n_sc - 1:
                softmax_b(units[idx - 1][0])
        emit_ones(pending)
        softmax_b(b_loc - 1)

    nc.compile()
    return nc


def make_in_maps(hidden, encoder_outputs, W_attn, b_attn, v, n_cores=8):
    hidden = np.asarray(hidden, dtype=np.float32)
    encoder_outputs = np.asarray(encoder_outputs, dtype=np.float32)
    W_attn = np.asarray(W_attn, dtype=np.float32)
    b_attn = np.asarray(b_attn, dtype=np.float32)
    v = np.asarray(v, dtype=np.float32)

    b = encoder_outputs.shape[0]
    b_loc = b // n_cores
    wt = np.ascontiguousarray(W_attn.T.astype(ml_dtypes.bfloat16))
    probe = np.array([PROBE_VALS], dtype=np.float32)
    # |att| <= sum|v| since |tanh| <= 1; shifting exp by this bound is
    # exact in infinite precision and keeps exp in (0, 1]
    nb = np.array([[-(np.abs(v).sum() + 1.0)]], dtype=np.float32)
    in_maps = []
    for i in range(n_cores):
        bsl = slice(b_loc * i, b_loc * (i + 1))
        in_maps.append({
            "wt": wt,
            "hiddenT": np.ascontiguousarray(
                hidden[0, bsl].T.astype(ml_dtypes.bfloat16)),
            "b_attn": b_attn,
            "v": v,
            "encT": np.ascontiguousarray(
                encoder_outputs[bsl].transpose(0, 2, 1)
                .astype(ml_dtypes.bfloat16)),
            "probe": probe,
            "nb": nb,
        })
    return in_maps


_NC_CACHE = {}


def _get_nc():
    if "nc" not in _NC_CACHE:
        _NC_CACHE["nc"] = build_nc(b_loc=4, s=2048, h=1024, n_cores=8)
    return _NC_CACHE["nc"]


def kernel(hidden, encoder_outputs, W_attn, b_attn, v):
    from concourse.bass_utils import run_bass_kernel_spmd

    nc = _get_nc()
    in_maps = make_in_maps(hidden, encoder_outputs, W_attn, b_attn, v,
                           n_cores=8)
    res = run_bass_kernel_spmd(nc, in_maps, core_ids=list(range(8)))
    out = np.concatenate([np.asarray(res.results[i]["out"])
                          for i in range(8)], axis=0)
    return out.astype(np.float32)
